# revision 12
# baseline (speedup 1.0000x reference)
"""Trainium2 Bass kernel for nn_Extractor (ray feature extraction / grid sample).

Pipeline per NeuronCore (8-way data parallel over (batch, row-block)):
  1. Stage a per-core subvolume of feature_volume into a row-addressable DRAM
     buffer (rows = 64-f32 z-runs, 256B stride, int16-addressable).
  2. Compute per-pixel ray geometry (coords, direction) on DVE/ACT, matching
     the reference's f32 op order.
  3. Compute per-sample ray_pts, idx (floor), and gather rows via the GPSIMD
     dma_gather ucode (1024 idxs/call); select the target voxel from each
     64-f32 row with an iota==offset mask + grouped reduce on DVE.
All shapes/geometry of the reference problem are hardcoded (b=2, 480x640,
X=256, 9 samples, depth in [1,4)).
"""
import numpy as np

B, H, W = 2, 480, 640
NPTS = 9
NSIDE = 4
VX = 256                      # feature volume extent
CORE_ROWS = 120               # pixel rows per core (real)
PROWS = 128                   # padded partition rows
CHUNK = 128                   # cols per chunk
NCHUNK = W // CHUNK           # 5
SAMP_PER_ROW = CHUNK * NPTS   # 1152 samples per partition per chunk
CALL_IDX = 1024               # dma_gather indices per call (ring/scratch limit)
CALLS_PER_GROUP = SAMP_PER_ROW // 64   # 18 (each call: 16 partitions x 64)
NGROUPS = PROWS // 16         # 8
ELEM = 64                     # f32 per gathered row (256B)
Z0 = 16                       # staged subvolume z origin
ZEXT = 128                    # staged z extent (2 rows of 64 per column)

_CACHE = {}


def _geom_constants(depth_max, intrinsics, extrinsics, origin, resolution):
    """Host-side per-batch constants, f32, matching reference op order."""
    import jax
    import jax.numpy as jnp
    K = np.asarray(intrinsics, np.float32)
    try:
        cpu = jax.devices("cpu")[0]
        with jax.default_device(cpu):
            Kinv = np.asarray(jnp.linalg.inv(jnp.asarray(K)), np.float32)
    except Exception:
        Kinv = np.linalg.inv(K).astype(np.float32)
    E = np.asarray(extrinsics, np.float32)
    org = np.asarray(origin, np.float32)
    res = np.float32(resolution)
    M = np.einsum("bij,bjk->bik", E[:, :3, :3], Kinv).astype(np.float32)
    t = E[:, :3, 3].astype(np.float32)                 # (b,3)
    inv_res = np.float32(1.0) / res
    eye_w = E[:, :3, 3]
    eye_v = ((eye_w - org[None, :]) / res).astype(np.float32)   # (b,3)
    return Kinv, M, t, org, res, inv_res, eye_v


def _center_bounds(Mb, tb, org, inv_res, rows, z_lo, z_hi):
    """Voxel-space bounds of center_v over (col, row, z) box, via corners
    (multilinear in (col,row,z)), +-(4 ray drift + 3 margin)."""
    cs = np.array([0.0, W - 1.0], np.float32)
    rs = np.array([rows[0], rows[1]], np.float32)
    zs = np.array([z_lo, z_hi], np.float32)
    pts = []
    for c in cs:
        for r in rs:
            for z in zs:
                p = Mb @ np.array([c * z, r * z, z], np.float32) + tb
                pts.append((p - org) * inv_res)
    pts = np.stack(pts)
    lo = pts.min(axis=0) - 7.0
    hi = pts.max(axis=0) + 7.0
    return lo, hi


def _build_program(XP, YP):
    """Build the SPMD Bass program (shapes only; per-core values via inputs)."""
    import concourse.bacc as bacc
    import concourse.mybir as mybir
    from concourse.tile import TileContext
    from concourse import library_config
    from concourse.alu_op_type import AluOpType as Op
    from concourse.bass import ds

    f32 = mybir.dt.float32
    i32 = mybir.dt.int32
    i16 = mybir.dt.int16
    AF = mybir.ActivationFunctionType

    nc = bacc.Bacc(None, target_bir_lowering=False, num_swdge_queues=4)

    depth = nc.declare_dram_parameter("depth", [PROWS, W], f32, isOutput=False)
    vol = nc.declare_dram_parameter("vol", [VX * VX * VX], f32, isOutput=False)
    # consts: packed per-core scalars [1, 32]
    cpak = nc.declare_dram_parameter("cpak", [PROWS, 32], f32, isOutput=False)
    colv = nc.declare_dram_parameter("colv", [PROWS, W], f32, isOutput=False)
    rowv = nc.declare_dram_parameter("rowv", [PROWS, 1], f32, isOutput=False)
    iotav = nc.declare_dram_parameter("iotav", [PROWS, ELEM], f32, isOutput=False)
    # staging AP offset (x0*65536 + y0*256 + Z0) as int32 [1,1]
    soff = nc.declare_dram_parameter("soff", [1, 1], i32, isOutput=False)

    o_ext = nc.declare_dram_parameter("ext", [PROWS, W * NPTS], f32, isOutput=True)
    o_rp = nc.declare_dram_parameter("rp", [PROWS, W * NPTS * 3], f32, isOutput=True)
    o_dir = nc.declare_dram_parameter("dirw", [PROWS, W * 3], f32, isOutput=True)
    o_dep = nc.declare_dram_parameter("dep", [PROWS, W], f32, isOutput=True)
    o_idx = nc.declare_dram_parameter("idxw", [PROWS, W * NPTS * 3], i32, isOutput=True)
    o_crd = nc.declare_dram_parameter("crd", [PROWS, W * 3], f32, isOutput=True)

    sub = nc.dram_tensor("sub", [XP * YP * ZEXT], f32)   # staged subvolume
    rowi_d = nc.dram_tensor("rowi_d", [PROWS, SAMP_PER_ROW], mybir.dt.int16)
    ot_d = nc.dram_tensor("ot_d", [PROWS, SAMP_PER_ROW], f32)

    # cpak slot map
    CP = dict(M00=0, M01=1, M02=2, M10=3, M11=4, M12=5, M20=6, M21=7, M22=8,
              T0=9, T1=10, T2=11, O0=12, O1=13, O2=14, IR=15,
              E0=16, E1=17, E2=18, X0=19, Y0=20, EPS=21)

    with TileContext(nc) as tc:
        nc.gpsimd.load_library(library_config.mlp)
        with tc.tile_pool(name="cons", bufs=1) as cons, \
             tc.tile_pool(name="work", bufs=1) as wk, \
             tc.tile_pool(name="gath", bufs=3) as gp:

            # ---- constants into SBUF, broadcast across partitions ----
            cp = cons.tile([PROWS, 32], f32)
            nc.sync.dma_start(out=cp[:], in_=cpak[:])
            colt = cons.tile([PROWS, W], f32)
            nc.sync.dma_start(out=colt[:], in_=colv[:])
            rowt = cons.tile([PROWS, 1], f32)
            nc.sync.dma_start(out=rowt[:], in_=rowv[:])
            iot = cons.tile([PROWS, ELEM], f32)
            nc.sync.dma_start(out=iot[:], in_=iotav[:])

            sofft = cons.tile([1, 1], i32)
            nc.sync.dma_start(out=sofft[:], in_=soff[:])
            soreg = nc.sync.value_load(sofft[:1, :1])

            def S(name):
                return cp[:, CP[name]:CP[name] + 1]

            # ---- stage subvolume: sub[x',y',z'] = vol[x0+x', y0+y', Z0+z'] ----
            vol_view = vol[ds(soreg, XP * 65536)].rearrange(
                "(x r) -> x r", x=XP)[:, :YP * 256].rearrange(
                "x (y r) -> x y r", y=YP)[:, :, :ZEXT]
            nc.sync.dma_start(
                out=sub[:].rearrange("(x y z) -> x y z", x=XP, y=YP),
                in_=vol_view)

            # depth passthrough
            nc.sync.dma_start(out=o_dep[:], in_=depth[:])

            sub_rows = sub[:].rearrange("(r e) -> r e", e=ELEM)

            for ci in range(NCHUNK):
                c0 = ci * CHUNK
                dz = wk.tile([PROWS, CHUNK], f32, tag="dz")
                nc.sync.dma_start(out=dz[:], in_=depth[:, c0:c0 + CHUNK])

                # ---- pixel stage ----
                colz = wk.tile([PROWS, CHUNK], f32, tag="colz")
                nc.vector.tensor_tensor(out=colz[:], in0=colt[:, c0:c0 + CHUNK],
                                        in1=dz[:], op=Op.mult)
                rowz = wk.tile([PROWS, CHUNK], f32, tag="rowz")
                nc.vector.scalar_tensor_tensor(out=rowz[:], in0=dz[:],
                                               scalar=rowt[:], in1=dz[:],
                                               op0=Op.mult, op1=Op.bypass)

                crd = wk.tile([PROWS, CHUNK * 3], f32, tag="crd")
                cen = wk.tile([PROWS, CHUNK * 3], f32, tag="cen")
                dirt = wk.tile([PROWS, CHUNK * 3], f32, tag="dirt")
                u = wk.tile([PROWS, CHUNK], f32, tag="u")
                for i in range(3):
                    Mi = [f"M{i}0", f"M{i}1", f"M{i}2"]
                    nc.vector.scalar_tensor_tensor(
                        out=u[:], in0=colz[:], scalar=S(Mi[0]), in1=colz[:],
                        op0=Op.mult, op1=Op.bypass)
                    nc.vector.scalar_tensor_tensor(
                        out=u[:], in0=rowz[:], scalar=S(Mi[1]), in1=u[:],
                        op0=Op.mult, op1=Op.add)
                    nc.vector.scalar_tensor_tensor(
                        out=u[:], in0=dz[:], scalar=S(Mi[2]), in1=u[:],
                        op0=Op.mult, op1=Op.add)
                    # coords = u + t_i
                    nc.vector.scalar_tensor_tensor(
                        out=crd[:, i::3], in0=u[:], scalar=S(f"T{i}"), in1=u[:],
                        op0=Op.add, op1=Op.bypass)
                    # center = (coords - origin_i) * inv_res
                    nc.vector.scalar_tensor_tensor(
                        out=u[:], in0=crd[:, i::3], scalar=S(f"O{i}"), in1=u[:],
                        op0=Op.subtract, op1=Op.bypass)
                    nc.vector.scalar_tensor_tensor(
                        out=cen[:, i::3], in0=u[:], scalar=S("IR"), in1=u[:],
                        op0=Op.mult, op1=Op.bypass)
                    # dir_un = center - eye_i
                    nc.vector.scalar_tensor_tensor(
                        out=dirt[:, i::3], in0=cen[:, i::3], scalar=S(f"E{i}"),
                        in1=u[:], op0=Op.subtract, op1=Op.bypass)

                # norm & normalize (matches ref: sqrt((dx2+dy2)+dz2), max(,1e-12), /)
                n2 = wk.tile([PROWS, CHUNK], f32, tag="n2")
                nc.vector.tensor_tensor(out=n2[:], in0=dirt[:, 0::3],
                                        in1=dirt[:, 0::3], op=Op.mult)
                nc.vector.scalar_tensor_tensor(
                    out=u[:], in0=dirt[:, 1::3], scalar=S("EPS"), in1=dirt[:, 1::3],
                    op0=Op.bypass, op1=Op.mult)
                nc.vector.tensor_tensor(out=n2[:], in0=n2[:], in1=u[:], op=Op.add)
                nc.vector.scalar_tensor_tensor(
                    out=u[:], in0=dirt[:, 2::3], scalar=S("EPS"), in1=dirt[:, 2::3],
                    op0=Op.bypass, op1=Op.mult)
                nc.vector.tensor_tensor(out=n2[:], in0=n2[:], in1=u[:], op=Op.add)
                nc.scalar.activation(out=n2[:], in_=n2[:], func=AF.Sqrt)
                nc.vector.scalar_tensor_tensor(
                    out=n2[:], in0=n2[:], scalar=S("EPS"), in1=n2[:],
                    op0=Op.max, op1=Op.bypass)
                nc.vector.reciprocal(out=n2[:], in_=n2[:])
                for i in range(3):
                    nc.vector.tensor_tensor(out=dirt[:, i::3], in0=dirt[:, i::3],
                                            in1=n2[:], op=Op.mult)

                nc.sync.dma_start(out=o_crd[:, c0 * 3:(c0 + CHUNK) * 3], in_=crd[:])
                nc.sync.dma_start(out=o_dir[:, c0 * 3:(c0 + CHUNK) * 3], in_=dirt[:])

                # ---- sample stage ----
                rp = wk.tile([PROWS, CHUNK * 27], f32, tag="rp")
                for k in range(NPTS):
                    off = float(k - NSIDE)
                    for i in range(3):
                        nc.vector.scalar_tensor_tensor(
                            out=rp[:, 3 * k + i::27], in0=dirt[:, i::3],
                            scalar=off, in1=cen[:, i::3],
                            op0=Op.mult, op1=Op.add)

                flr = wk.tile([PROWS, CHUNK * 27], f32, tag="flr")
                idxt = wk.tile([PROWS, CHUNK * 27], i32, tag="idxt")
                gtm = wk.tile([PROWS, CHUNK * 27], f32, tag="gtm")
                nc.vector.tensor_copy(out=idxt[:], in_=rp[:])      # rint
                nc.vector.tensor_copy(out=flr[:], in_=idxt[:])     # back to f32
                nc.vector.tensor_tensor(out=gtm[:], in0=flr[:], in1=rp[:],
                                        op=Op.is_gt)
                nc.vector.tensor_tensor(out=flr[:], in0=flr[:], in1=gtm[:],
                                        op=Op.subtract)
                nc.vector.tensor_copy(out=idxt[:], in_=flr[:])
                nc.sync.dma_start(out=o_rp[:, c0 * 27:(c0 + CHUNK) * 27], in_=rp[:])
                nc.sync.dma_start(out=o_idx[:, c0 * 27:(c0 + CHUNK) * 27], in_=idxt[:])

                # ---- row index & z offset (f32 exact int arithmetic) ----
                # o = mod(fz - Z0, 64); zq = (fz - Z0 - o)/64
                # row = ((fx - x0)*YP + (fy - y0))*2 + zq
                ot = wk.tile([PROWS, SAMP_PER_ROW], f32, tag="ot")
                t1 = wk.tile([PROWS, SAMP_PER_ROW], f32, tag="t1")
                t2 = wk.tile([PROWS, SAMP_PER_ROW], f32, tag="t2")
                fx, fy, fz = flr[:, 0::3], flr[:, 1::3], flr[:, 2::3]
                # t2 = z' = fz - Z0 ;  zq = (z' >= 64) ;  o = z' - 64*zq
                nc.vector.tensor_scalar(out=t2[:], in0=fz, scalar1=float(Z0),
                                        scalar2=None, op0=Op.subtract)
                zq = wk.tile([PROWS, SAMP_PER_ROW], f32, tag="zq")
                nc.vector.tensor_scalar(out=zq[:], in0=t2[:], scalar1=64.0,
                                        scalar2=None, op0=Op.is_ge)
                nc.vector.scalar_tensor_tensor(out=t1[:], in0=zq[:], scalar=-64.0,
                                               in1=t2[:], op0=Op.mult, op1=Op.add)
                nc.vector.tensor_copy(
                    out=ot[:].rearrange("p (q8 b c8) -> p b c8 q8", q8=8, c8=8),
                    in_=t1[:])
                nc.vector.tensor_copy(out=t2[:], in_=zq[:])
                # t1 = (fx - x0) * (2*YP)
                nc.vector.scalar_tensor_tensor(
                    out=t1[:], in0=fx, scalar=S("X0"), in1=fx,
                    op0=Op.subtract, op1=Op.bypass)
                nc.vector.tensor_scalar(out=t1[:], in0=t1[:], scalar1=float(2 * YP),
                                        scalar2=None, op0=Op.mult)
                # t1 += (fy - y0) * 2
                rowf = wk.tile([PROWS, SAMP_PER_ROW], f32, tag="rowf")
                nc.vector.scalar_tensor_tensor(
                    out=rowf[:], in0=fy, scalar=S("Y0"), in1=fy,
                    op0=Op.subtract, op1=Op.bypass)
                nc.vector.tensor_scalar(out=rowf[:], in0=rowf[:], scalar1=2.0,
                                        scalar2=None, op0=Op.mult)
                nc.vector.tensor_tensor(out=t1[:], in0=t1[:], in1=rowf[:], op=Op.add)
                nc.vector.tensor_tensor(out=t1[:], in0=t1[:], in1=t2[:], op=Op.add)
                rowi = wk.tile([PROWS, SAMP_PER_ROW], i16, tag="rowi")
                nc.vector.tensor_copy(out=rowi[:], in_=t1[:])

                # ---- replicate row indices for dma_gather (via DRAM) ----
                # irep[q, g*1152 + j] = rowi[16g + q%16, j]
                nc.sync.dma_start(out=rowi_d[:], in_=rowi[:])
                nc.sync.dma_start(out=ot_d[:], in_=ot[:])
                irep = wk.tile([PROWS, NGROUPS * SAMP_PER_ROW], i16, tag="irep")
                nc.vector.memset(irep[:], 0)
                for g in range(NGROUPS):
                    nc.sync.dma_start(
                        out=irep[:, g * SAMP_PER_ROW:(g + 1) * SAMP_PER_ROW],
                        in_=rowi_d[16 * g:16 * g + 16, :][None]
                            .to_broadcast([8, 16, SAMP_PER_ROW]))

                # ---- o permute per group:  operm[q, b*8+r] = ot[16g+q%16, 64b+8*(q//16)+r]
                for g in range(NGROUPS):
                    operm = wk.tile([PROWS, CALLS_PER_GROUP * 8], f32,
                                    tag="operm", name=f"operm_{ci}_{g}")
                    nc.vector.memset(operm[:], 0)
                    nc.sync.dma_start(
                        out=operm[:],
                        in_=ot_d[16 * g:16 * g + 16, :].rearrange(
                            "p (q8 f) -> q8 p f", q8=8))

                    selg = wk.tile([PROWS, CALLS_PER_GROUP * 8], f32,
                                   tag="selg", name=f"selg_{ci}_{g}")
                    for b in range(CALLS_PER_GROUP):
                        gt = gp.tile([PROWS, 8, ELEM], f32, tag="gt")
                        nc.gpsimd.dma_gather(
                            out_ap=gt[:],
                            in_ap=sub_rows,
                            idxs_ap=irep[:, g * SAMP_PER_ROW + b * 64:
                                         g * SAMP_PER_ROW + (b + 1) * 64],
                            num_idxs=CALL_IDX, num_idxs_reg=CALL_IDX,
                            elem_size=ELEM, elem_step=ELEM,
                            queue_num=(g * CALLS_PER_GROUP + b) % 4)
                        mask = gp.tile([PROWS, 8, ELEM], f32, tag="mask")
                        nc.vector.tensor_tensor(
                            out=mask[:],
                            in0=iot[:].rearrange("p (a j) -> p a j", a=1)
                                .to_broadcast([PROWS, 8, ELEM]),
                            in1=operm[:, b * 8:(b + 1) * 8]
                                .rearrange("p (f a) -> p f a", a=1)
                                .to_broadcast([PROWS, 8, ELEM]),
                            op=Op.is_equal)
                        nc.vector.tensor_tensor(out=mask[:], in0=mask[:],
                                                in1=gt[:], op=Op.mult)
                        nc.vector.tensor_reduce(
                            out=selg[:, b * 8:(b + 1) * 8]
                                .rearrange("p (f a) -> p f a", a=1),
                            in_=mask[:], axis=mybir.AxisListType.X, op=Op.add)

                    # dump selg; layout [q, (ci, g, b, c8)], host unpermutes
                    nc.sync.dma_start(
                        out=o_ext[:, (ci * NGROUPS + g) * 144:
                                  (ci * NGROUPS + g + 1) * 144],
                        in_=selg[:])

    nc.finalize()
    return nc


def _get_program(XP, YP):
    key = (XP, YP)
    if key not in _CACHE:
        _CACHE[key] = _build_program(XP, YP)
    return _CACHE[key]


def _prepare_core_inputs(core, depth, vol_flat, Kinv, M, t, org, res, inv_res,
                         eye_v, XP, YP, x0s, y0s):
    b = core // 4
    r0 = CORE_ROWS * (core % 4)
    dshard = np.zeros((PROWS, W), np.float32)
    dshard[:CORE_ROWS] = depth[b, r0:r0 + CORE_ROWS]
    dshard[CORE_ROWS:] = 1.0
    cp = np.zeros((1, 32), np.float32)
    Mb = M[b]
    cp[0, 0:9] = Mb.reshape(-1)
    cp[0, 9:12] = t[b]
    cp[0, 12:15] = org
    cp[0, 15] = inv_res
    cp[0, 16:19] = eye_v[b]
    cp[0, 19] = np.float32(x0s[core])
    cp[0, 20] = np.float32(y0s[core])
    cp[0, 21] = np.float32(1e-12)
    colvv = np.tile(np.arange(W, dtype=np.float32)[None, :], (PROWS, 1))
    rowvv = np.zeros((PROWS, 1), np.float32)
    rowvv[:CORE_ROWS, 0] = r0 + np.arange(CORE_ROWS, dtype=np.float32)
    iotavv = np.tile(np.arange(ELEM, dtype=np.float32)[None, :], (PROWS, 1))
    soffv = np.array([[x0s[core] * 65536 + y0s[core] * 256 + Z0]], np.int32)
    return {
        "depth": dshard, "vol": vol_flat, "cpak": np.tile(cp, (PROWS, 1)),
        "colv": colvv, "rowv": rowvv, "iotav": iotavv, "soff": soffv,
    }


def kernel(depth, extrinsics, intrinsics, feature_volume, origin, resolution):
    depth = np.asarray(depth, np.float32)
    vol = np.ascontiguousarray(np.asarray(feature_volume, np.float32))
    Kinv, M, t, org, res, inv_res, eye_v = _geom_constants(
        4.0, intrinsics, extrinsics, origin, resolution)

    # per-core staging bounds (shared XP/YP dims = max over cores)
    x0s, y0s = [], []
    spans = []
    for core in range(8):
        b = core // 4
        r0 = CORE_ROWS * (core % 4)
        lo, hi = _center_bounds(M[b], t[b], org, inv_res,
                                (r0, r0 + CORE_ROWS - 1.0), 1.0, 4.0)
        x0 = int(max(0, np.floor(lo[0])))
        y0 = int(max(0, np.floor(lo[1])))
        x1 = int(min(VX, np.ceil(hi[0]) + 1))
        y1 = int(min(VX, np.ceil(hi[1]) + 1))
        x0s.append(x0)
        y0s.append(y0)
        spans.append((x1 - x0, y1 - y0))
    XP = max(s[0] for s in spans)
    YP = max(s[1] for s in spans)
    # clamp origins so x0+XP <= VX
    x0s = [min(x0, VX - XP) for x0 in x0s]
    y0s = [min(y0, VX - YP) for y0 in y0s]
    assert XP * YP * 2 <= 32767, (XP, YP)

    nc = _get_program(XP, YP)

    vol_flat = vol.reshape(-1)
    in_maps = [_prepare_core_inputs(c, depth, vol_flat, Kinv, M, t, org, res,
                                    inv_res, eye_v, XP, YP, x0s, y0s)
               for c in range(8)]

    from concourse.bass_utils import run_bass_kernel_spmd
    res8 = run_bass_kernel_spmd(nc, in_maps, list(range(8)))

    n = H * W
    ext = np.empty((B, n, NPTS), np.float32)
    rp = np.empty((B, n, NPTS, 3), np.float32)
    dirw = np.empty((B, n, 3), np.float32)
    dep = np.empty((B, n), np.float32)
    idxw = np.empty((B, n, NPTS, 3), np.int32)
    crd = np.empty((B, n, 3), np.float32)
    for core in range(8):
        b = core // 4
        r0 = CORE_ROWS * (core % 4)
        sl = slice(r0 * W, (r0 + CORE_ROWS) * W)
        r = res8.results[core]
        ep = r["ext"].reshape(8, 16, NCHUNK, NGROUPS, 18, 8)
        ep = ep.transpose(3, 1, 2, 4, 5, 0).reshape(PROWS, NCHUNK * SAMP_PER_ROW)
        ext[b, sl] = ep[:CORE_ROWS].reshape(CORE_ROWS * W, NPTS)
        rp[b, sl] = r["rp"][:CORE_ROWS].reshape(CORE_ROWS * W, NPTS, 3)
        dirw[b, sl] = r["dirw"][:CORE_ROWS].reshape(CORE_ROWS * W, 3)
        dep[b, sl] = r["dep"][:CORE_ROWS].reshape(CORE_ROWS * W)
        idxw[b, sl] = r["idxw"][:CORE_ROWS].reshape(CORE_ROWS * W, NPTS, 3)
        crd[b, sl] = r["crd"][:CORE_ROWS].reshape(CORE_ROWS * W, 3)
    return ext, rp, dirw, dep, idxw, crd


# revision 17
# speedup vs baseline: 90.2127x; 90.2127x over previous
"""Trainium2 Bass kernel for nn_Extractor (ray feature extraction / grid sample).

Pipeline per NeuronCore (8-way data parallel over (batch, row-block)):
  1. Stage a per-core subvolume of feature_volume into a row-addressable DRAM
     buffer (rows = 64-f32 z-runs, 256B stride, int16-addressable).
  2. Compute per-pixel ray geometry (coords, direction) on DVE/ACT, matching
     the reference's f32 op order.
  3. Compute per-sample ray_pts, idx (floor), and gather rows via the GPSIMD
     dma_gather ucode (1024 idxs/call); select the target voxel from each
     64-f32 row with an iota==offset mask + grouped reduce on DVE.
All shapes/geometry of the reference problem are hardcoded (b=2, 480x640,
X=256, 9 samples, depth in [1,4)).
"""
import os
import numpy as np

SIM_MODE = os.environ.get("KERNEL_SIM") == "1"

B, H, W = 2, 480, 640
NPTS = 9
NSIDE = 4
VX = 256                      # feature volume extent
CORE_ROWS = 120               # pixel rows per core (real)
PROWS = 128                   # padded partition rows
CHUNK = 128                   # cols per chunk
NCHUNK = W // CHUNK           # 5
SAMP_PER_ROW = CHUNK * NPTS   # 1152 samples per partition per chunk
CALL_IDX = 1024               # dma_gather indices per call (HW ring/scratch limit)
CALLS_PER_GROUP = SAMP_PER_ROW // (CALL_IDX // 16)   # 12 calls of 96 samples/partition
COLS8 = CALL_IDX // 128       # 12 out cols per call
NGROUPS = PROWS // 16         # 8
ELEM = 64                     # f32 per gathered row (256B)
Z0 = 16                       # staged subvolume z origin
ZEXT = 128                    # staged z extent (2 rows of 64 per column)

_CACHE = {}


def _geom_constants(depth_max, intrinsics, extrinsics, origin, resolution):
    """Host-side per-batch constants, f32, matching reference op order."""
    import jax
    import jax.numpy as jnp
    K = np.asarray(intrinsics, np.float32)
    try:
        cpu = jax.devices("cpu")[0]
        with jax.default_device(cpu):
            Kinv = np.asarray(jnp.linalg.inv(jnp.asarray(K)), np.float32)
    except Exception:
        Kinv = np.linalg.inv(K).astype(np.float32)
    E = np.asarray(extrinsics, np.float32)
    org = np.asarray(origin, np.float32)
    res = np.float32(resolution)
    M = np.einsum("bij,bjk->bik", E[:, :3, :3], Kinv).astype(np.float32)
    t = E[:, :3, 3].astype(np.float32)                 # (b,3)
    inv_res = np.float32(1.0) / res
    eye_w = E[:, :3, 3]
    eye_v = ((eye_w - org[None, :]) / res).astype(np.float32)   # (b,3)
    return Kinv, M, t, org, res, inv_res, eye_v


def _center_bounds(Mb, tb, org, inv_res, rows, z_lo, z_hi):
    """Voxel-space bounds of center_v over (col, row, z) box, via corners
    (multilinear in (col,row,z)), +-(4 ray drift + 3 margin)."""
    cs = np.array([0.0, W - 1.0], np.float32)
    rs = np.array([rows[0], rows[1]], np.float32)
    zs = np.array([z_lo, z_hi], np.float32)
    pts = []
    for c in cs:
        for r in rs:
            for z in zs:
                p = Mb @ np.array([c * z, r * z, z], np.float32) + tb
                pts.append((p - org) * inv_res)
    pts = np.stack(pts)
    lo = pts.min(axis=0) - 7.0
    hi = pts.max(axis=0) + 7.0
    return lo, hi


def _build_program(XP, YP):
    """Build the SPMD Bass program (shapes only; per-core values via inputs)."""
    import concourse.bacc as bacc
    import concourse.mybir as mybir
    from concourse.tile import TileContext
    from concourse import library_config
    from concourse.alu_op_type import AluOpType as Op
    from concourse.bass import ds

    f32 = mybir.dt.float32
    i32 = mybir.dt.int32
    i16 = mybir.dt.int16
    AF = mybir.ActivationFunctionType

    nc = bacc.Bacc(None, target_bir_lowering=False, num_swdge_queues=4)

    depth = nc.declare_dram_parameter("depth", [PROWS, W], f32, isOutput=False)
    vol = nc.declare_dram_parameter("vol", [VX * VX * VX], f32, isOutput=False)
    # consts: packed per-core scalars [1, 32]
    cpak = nc.declare_dram_parameter("cpak", [PROWS, 32], f32, isOutput=False)
    colv = nc.declare_dram_parameter("colv", [PROWS, W], f32, isOutput=False)
    rowv = nc.declare_dram_parameter("rowv", [PROWS, 1], f32, isOutput=False)
    iotav = nc.declare_dram_parameter("iotav", [PROWS, ELEM], f32, isOutput=False)
    # staging AP offset (x0*65536 + y0*256 + Z0) as int32 [1,1]
    soff = nc.declare_dram_parameter("soff", [1, 1], i32, isOutput=False)

    o_ext = nc.declare_dram_parameter("ext", [PROWS, W * NPTS], f32, isOutput=True)
    o_rp = nc.declare_dram_parameter("rp", [PROWS, W * NPTS * 3], f32, isOutput=True)
    o_dir = nc.declare_dram_parameter("dirw", [PROWS, W * 3], f32, isOutput=True)
    o_dep = nc.declare_dram_parameter("dep", [PROWS, W], f32, isOutput=True)
    o_idx = nc.declare_dram_parameter("idxw", [PROWS, W * NPTS * 3], i32, isOutput=True)
    o_crd = nc.declare_dram_parameter("crd", [PROWS, W * 3], f32, isOutput=True)

    sub = nc.dram_tensor("sub", [XP * YP * ZEXT], f32)   # staged subvolume
    rowi_d = nc.dram_tensor("rowi_d", [PROWS, SAMP_PER_ROW], mybir.dt.int16)
    ot_d = nc.dram_tensor("ot_d", [PROWS, SAMP_PER_ROW], f32)

    # cpak slot map
    CP = dict(M00=0, M01=1, M02=2, M10=3, M11=4, M12=5, M20=6, M21=7, M22=8,
              T0=9, T1=10, T2=11, O0=12, O1=13, O2=14, IR=15,
              E0=16, E1=17, E2=18, X0=19, Y0=20, EPS=21)

    with TileContext(nc) as tc:
        nc.gpsimd.load_library(library_config.mlp)
        with tc.tile_pool(name="cons", bufs=1) as cons, \
             tc.tile_pool(name="work", bufs=1) as wk, \
             tc.tile_pool(name="gath", bufs=3) as gp:

            # ---- constants into SBUF, broadcast across partitions ----
            cp = cons.tile([PROWS, 32], f32)
            nc.sync.dma_start(out=cp[:], in_=cpak[:])
            colt = cons.tile([PROWS, W], f32)
            nc.sync.dma_start(out=colt[:], in_=colv[:])
            rowt = cons.tile([PROWS, 1], f32)
            nc.sync.dma_start(out=rowt[:], in_=rowv[:])
            iot = cons.tile([PROWS, ELEM], f32)
            nc.sync.dma_start(out=iot[:], in_=iotav[:])

            sofft = cons.tile([1, 1], i32)
            nc.sync.dma_start(out=sofft[:], in_=soff[:])
            soreg = nc.sync.value_load(sofft[:1, :1])

            def S(name):
                return cp[:, CP[name]:CP[name] + 1]

            # ---- stage subvolume: sub[x',y',z'] = vol[x0+x', y0+y', Z0+z'] ----
            vol_view = vol[ds(soreg, XP * 65536)].rearrange(
                "(x r) -> x r", x=XP)[:, :YP * 256].rearrange(
                "x (y r) -> x y r", y=YP)[:, :, :ZEXT]
            nc.sync.dma_start(
                out=sub[:].rearrange("(x y z) -> x y z", x=XP, y=YP),
                in_=vol_view)

            # depth passthrough
            nc.sync.dma_start(out=o_dep[:], in_=depth[:])

            sub_rows = sub[:].rearrange("(r e) -> r e", e=ELEM)

            for ci in range(NCHUNK):
                c0 = ci * CHUNK
                dz = wk.tile([PROWS, CHUNK], f32, tag="dz")
                nc.sync.dma_start(out=dz[:], in_=depth[:, c0:c0 + CHUNK])

                # ---- pixel stage ----
                colz = wk.tile([PROWS, CHUNK], f32, tag="colz")
                nc.vector.tensor_tensor(out=colz[:], in0=colt[:, c0:c0 + CHUNK],
                                        in1=dz[:], op=Op.mult)
                rowz = wk.tile([PROWS, CHUNK], f32, tag="rowz")
                nc.vector.scalar_tensor_tensor(out=rowz[:], in0=dz[:],
                                               scalar=rowt[:], in1=dz[:],
                                               op0=Op.mult, op1=Op.bypass)

                crd = wk.tile([PROWS, CHUNK * 3], f32, tag="crd")
                cen = wk.tile([PROWS, CHUNK * 3], f32, tag="cen")
                dirt = wk.tile([PROWS, CHUNK * 3], f32, tag="dirt")
                u = wk.tile([PROWS, CHUNK], f32, tag="u")
                for i in range(3):
                    Mi = [f"M{i}0", f"M{i}1", f"M{i}2"]
                    nc.vector.scalar_tensor_tensor(
                        out=u[:], in0=colz[:], scalar=S(Mi[0]), in1=colz[:],
                        op0=Op.mult, op1=Op.bypass)
                    nc.vector.scalar_tensor_tensor(
                        out=u[:], in0=rowz[:], scalar=S(Mi[1]), in1=u[:],
                        op0=Op.mult, op1=Op.add)
                    nc.vector.scalar_tensor_tensor(
                        out=u[:], in0=dz[:], scalar=S(Mi[2]), in1=u[:],
                        op0=Op.mult, op1=Op.add)
                    # coords = u + t_i
                    nc.vector.scalar_tensor_tensor(
                        out=crd[:, i::3], in0=u[:], scalar=S(f"T{i}"), in1=u[:],
                        op0=Op.add, op1=Op.bypass)
                    # center = (coords - origin_i) * inv_res
                    nc.vector.scalar_tensor_tensor(
                        out=u[:], in0=crd[:, i::3], scalar=S(f"O{i}"), in1=u[:],
                        op0=Op.subtract, op1=Op.bypass)
                    nc.vector.scalar_tensor_tensor(
                        out=cen[:, i::3], in0=u[:], scalar=S("IR"), in1=u[:],
                        op0=Op.mult, op1=Op.bypass)
                    # dir_un = center - eye_i
                    nc.vector.scalar_tensor_tensor(
                        out=dirt[:, i::3], in0=cen[:, i::3], scalar=S(f"E{i}"),
                        in1=u[:], op0=Op.subtract, op1=Op.bypass)

                # norm & normalize (matches ref: sqrt((dx2+dy2)+dz2), max(,1e-12), /)
                n2 = wk.tile([PROWS, CHUNK], f32, tag="n2")
                nc.vector.tensor_tensor(out=n2[:], in0=dirt[:, 0::3],
                                        in1=dirt[:, 0::3], op=Op.mult)
                nc.vector.scalar_tensor_tensor(
                    out=u[:], in0=dirt[:, 1::3], scalar=S("EPS"), in1=dirt[:, 1::3],
                    op0=Op.bypass, op1=Op.mult)
                nc.vector.tensor_tensor(out=n2[:], in0=n2[:], in1=u[:], op=Op.add)
                nc.vector.scalar_tensor_tensor(
                    out=u[:], in0=dirt[:, 2::3], scalar=S("EPS"), in1=dirt[:, 2::3],
                    op0=Op.bypass, op1=Op.mult)
                nc.vector.tensor_tensor(out=n2[:], in0=n2[:], in1=u[:], op=Op.add)
                nc.scalar.activation(out=n2[:], in_=n2[:], func=AF.Sqrt)
                nc.vector.scalar_tensor_tensor(
                    out=n2[:], in0=n2[:], scalar=S("EPS"), in1=n2[:],
                    op0=Op.max, op1=Op.bypass)
                rcp = wk.tile([PROWS, CHUNK], f32, tag="rcp")
                nc.vector.reciprocal(out=rcp[:], in_=n2[:])
                # Newton: r1 = r0*(2 - m*r0)  (one step towards the IEEE quotient)
                nc.vector.tensor_tensor(out=u[:], in0=n2[:], in1=rcp[:], op=Op.mult)
                nc.vector.tensor_scalar(out=u[:], in0=u[:], scalar1=-1.0,
                                        scalar2=2.0, op0=Op.mult, op1=Op.add)
                nc.vector.tensor_tensor(out=rcp[:], in0=rcp[:], in1=u[:], op=Op.mult)
                for i in range(3):
                    nc.vector.tensor_tensor(out=dirt[:, i::3], in0=dirt[:, i::3],
                                            in1=rcp[:], op=Op.mult)

                nc.sync.dma_start(out=o_crd[:, c0 * 3:(c0 + CHUNK) * 3], in_=crd[:])
                nc.sync.dma_start(out=o_dir[:, c0 * 3:(c0 + CHUNK) * 3], in_=dirt[:])

                # ---- sample stage ----
                rp = wk.tile([PROWS, CHUNK * 27], f32, tag="rp")
                for k in range(NPTS):
                    off = float(k - NSIDE)
                    for i in range(3):
                        nc.vector.scalar_tensor_tensor(
                            out=rp[:, 3 * k + i::27], in0=dirt[:, i::3],
                            scalar=off, in1=cen[:, i::3],
                            op0=Op.mult, op1=Op.add)

                flr = wk.tile([PROWS, CHUNK * 27], f32, tag="flr")
                idxt = wk.tile([PROWS, CHUNK * 27], i32, tag="idxt")
                gtm = wk.tile([PROWS, CHUNK * 27], f32, tag="gtm")
                nc.scalar.activation(out=idxt[:], in_=rp[:], func=AF.Copy)  # rint
                nc.scalar.activation(out=flr[:], in_=idxt[:], func=AF.Copy)
                nc.vector.tensor_tensor(out=gtm[:], in0=flr[:], in1=rp[:],
                                        op=Op.is_gt)
                nc.vector.tensor_tensor(out=flr[:], in0=flr[:], in1=gtm[:],
                                        op=Op.subtract)
                nc.vector.tensor_copy(out=idxt[:], in_=flr[:])
                nc.sync.dma_start(out=o_rp[:, c0 * 27:(c0 + CHUNK) * 27], in_=rp[:])
                nc.sync.dma_start(out=o_idx[:, c0 * 27:(c0 + CHUNK) * 27], in_=idxt[:])

                # ---- row index & z offset (f32 exact int arithmetic) ----
                # o = mod(fz - Z0, 64); zq = (fz - Z0 - o)/64
                # row = ((fx - x0)*YP + (fy - y0))*2 + zq
                ot = wk.tile([PROWS, SAMP_PER_ROW], f32, tag="ot")
                t1 = wk.tile([PROWS, SAMP_PER_ROW], f32, tag="t1")
                t2 = wk.tile([PROWS, SAMP_PER_ROW], f32, tag="t2")
                fx, fy, fz = flr[:, 0::3], flr[:, 1::3], flr[:, 2::3]
                # t2 = z' = fz - Z0 ;  zq = (z' >= 64) ;  o = z' - 64*zq
                nc.vector.tensor_scalar(out=t2[:], in0=fz, scalar1=float(Z0),
                                        scalar2=None, op0=Op.subtract)
                zq = wk.tile([PROWS, SAMP_PER_ROW], f32, tag="zq")
                nc.vector.tensor_scalar(out=zq[:], in0=t2[:], scalar1=64.0,
                                        scalar2=None, op0=Op.is_ge)
                nc.vector.scalar_tensor_tensor(out=t1[:], in0=zq[:], scalar=-64.0,
                                               in1=t2[:], op0=Op.mult, op1=Op.add)
                nc.vector.tensor_copy(
                    out=ot[:].rearrange("p (q8 b c8) -> p b c8 q8", q8=8, c8=COLS8),
                    in_=t1[:])
                nc.vector.tensor_copy(out=t2[:], in_=zq[:])
                # t1 = (fx - x0) * (2*YP)
                nc.vector.scalar_tensor_tensor(
                    out=t1[:], in0=fx, scalar=S("X0"), in1=fx,
                    op0=Op.subtract, op1=Op.bypass)
                nc.vector.tensor_scalar(out=t1[:], in0=t1[:], scalar1=float(2 * YP),
                                        scalar2=None, op0=Op.mult)
                # t1 += (fy - y0) * 2
                rowf = wk.tile([PROWS, SAMP_PER_ROW], f32, tag="rowf")
                nc.vector.scalar_tensor_tensor(
                    out=rowf[:], in0=fy, scalar=S("Y0"), in1=fy,
                    op0=Op.subtract, op1=Op.bypass)
                nc.vector.tensor_scalar(out=rowf[:], in0=rowf[:], scalar1=2.0,
                                        scalar2=None, op0=Op.mult)
                nc.vector.tensor_tensor(out=t1[:], in0=t1[:], in1=rowf[:], op=Op.add)
                nc.vector.tensor_tensor(out=t1[:], in0=t1[:], in1=t2[:], op=Op.add)
                rowi = wk.tile([PROWS, SAMP_PER_ROW], i16, tag="rowi")
                nc.vector.tensor_copy(out=rowi[:], in_=t1[:])

                # ---- replicate row indices for dma_gather (via DRAM) ----
                # irep[q, g*1152 + j] = rowi[16g + q%16, j]
                nc.sync.dma_start(out=rowi_d[:], in_=rowi[:])
                nc.sync.dma_start(out=ot_d[:], in_=ot[:])
                irep = wk.tile([PROWS, NGROUPS * SAMP_PER_ROW], i16, tag="irep")
                if SIM_MODE:
                    nc.vector.memset(irep[:], 0)
                for g in range(NGROUPS):
                    nc.sync.dma_start(
                        out=irep[:, g * SAMP_PER_ROW:(g + 1) * SAMP_PER_ROW],
                        in_=rowi_d[16 * g:16 * g + 16, :][None]
                            .to_broadcast([8, 16, SAMP_PER_ROW]))

                # ---- o permute per group:  operm[q, b*8+r] = ot[16g+q%16, 64b+8*(q//16)+r]
                for g in range(NGROUPS):
                    operm = wk.tile([PROWS, CALLS_PER_GROUP * COLS8], f32,
                                    tag="operm", name=f"operm_{ci}_{g}")
                    if SIM_MODE:
                        nc.vector.memset(operm[:], 0)
                    nc.sync.dma_start(
                        out=operm[:],
                        in_=ot_d[16 * g:16 * g + 16, :].rearrange(
                            "p (q8 f) -> q8 p f", q8=8))

                    selg = wk.tile([PROWS, CALLS_PER_GROUP * COLS8], f32,
                                   tag="selg", name=f"selg_{ci}_{g}")
                    for b in range(CALLS_PER_GROUP):
                        gt = gp.tile([PROWS, COLS8, ELEM], f32, tag="gt")
                        nc.gpsimd.dma_gather(
                            out_ap=gt[:],
                            in_ap=sub_rows,
                            idxs_ap=irep[:, g * SAMP_PER_ROW + b * (CALL_IDX // 16):
                                         g * SAMP_PER_ROW + (b + 1) * (CALL_IDX // 16)],
                            num_idxs=CALL_IDX, num_idxs_reg=CALL_IDX,
                            elem_size=ELEM, elem_step=ELEM,
                            queue_num=(g * CALLS_PER_GROUP + b) % 4)
                        mask = gp.tile([PROWS, COLS8, ELEM], f32, tag="mask")
                        nc.vector.tensor_tensor(
                            out=mask[:],
                            in0=iot[:].rearrange("p (a j) -> p a j", a=1)
                                .to_broadcast([PROWS, COLS8, ELEM]),
                            in1=operm[:, b * COLS8:(b + 1) * COLS8]
                                .rearrange("p (f a) -> p f a", a=1)
                                .to_broadcast([PROWS, COLS8, ELEM]),
                            op=Op.is_equal)
                        nc.vector.tensor_tensor(out=mask[:], in0=mask[:],
                                                in1=gt[:], op=Op.mult)
                        nc.vector.tensor_reduce(
                            out=selg[:, b * COLS8:(b + 1) * COLS8]
                                .rearrange("p (f a) -> p f a", a=1),
                            in_=mask[:], axis=mybir.AxisListType.X, op=Op.add)

                    # dump selg; layout [q, (ci, g, b, c8)], host unpermutes
                    nc.sync.dma_start(
                        out=o_ext[:, (ci * NGROUPS + g) * 144:
                                  (ci * NGROUPS + g + 1) * 144],
                        in_=selg[:])

    nc.finalize()
    return nc


def _get_program(XP, YP):
    key = (XP, YP)
    if key not in _CACHE:
        _CACHE[key] = _build_program(XP, YP)
    return _CACHE[key]


def _prepare_core_inputs(core, depth, vol_flat, Kinv, M, t, org, res, inv_res,
                         eye_v, XP, YP, x0s, y0s):
    b = core // 4
    r0 = CORE_ROWS * (core % 4)
    dshard = np.zeros((PROWS, W), np.float32)
    dshard[:CORE_ROWS] = depth[b, r0:r0 + CORE_ROWS]
    dshard[CORE_ROWS:] = 1.0
    cp = np.zeros((1, 32), np.float32)
    Mb = M[b]
    cp[0, 0:9] = Mb.reshape(-1)
    cp[0, 9:12] = t[b]
    cp[0, 12:15] = org
    cp[0, 15] = inv_res
    cp[0, 16:19] = eye_v[b]
    cp[0, 19] = np.float32(x0s[core])
    cp[0, 20] = np.float32(y0s[core])
    cp[0, 21] = np.float32(1e-12)
    colvv = np.tile(np.arange(W, dtype=np.float32)[None, :], (PROWS, 1))
    rowvv = np.zeros((PROWS, 1), np.float32)
    rowvv[:CORE_ROWS, 0] = r0 + np.arange(CORE_ROWS, dtype=np.float32)
    iotavv = np.tile(np.arange(ELEM, dtype=np.float32)[None, :], (PROWS, 1))
    soffv = np.array([[x0s[core] * 65536 + y0s[core] * 256 + Z0]], np.int32)
    return {
        "depth": dshard, "vol": vol_flat, "cpak": np.tile(cp, (PROWS, 1)),
        "colv": colvv, "rowv": rowvv, "iotav": iotavv, "soff": soffv,
    }


def kernel(depth, extrinsics, intrinsics, feature_volume, origin, resolution):
    depth = np.asarray(depth, np.float32)
    vol = np.ascontiguousarray(np.asarray(feature_volume, np.float32))
    Kinv, M, t, org, res, inv_res, eye_v = _geom_constants(
        4.0, intrinsics, extrinsics, origin, resolution)

    # per-core staging bounds (shared XP/YP dims = max over cores)
    x0s, y0s = [], []
    spans = []
    for core in range(8):
        b = core // 4
        r0 = CORE_ROWS * (core % 4)
        lo, hi = _center_bounds(M[b], t[b], org, inv_res,
                                (r0, r0 + CORE_ROWS - 1.0), 1.0, 4.0)
        x0 = int(max(0, np.floor(lo[0])))
        y0 = int(max(0, np.floor(lo[1])))
        x1 = int(min(VX, np.ceil(hi[0]) + 1))
        y1 = int(min(VX, np.ceil(hi[1]) + 1))
        x0s.append(x0)
        y0s.append(y0)
        spans.append((x1 - x0, y1 - y0))
    XP = max(s[0] for s in spans)
    YP = max(s[1] for s in spans)
    # clamp origins so x0+XP <= VX
    x0s = [min(x0, VX - XP) for x0 in x0s]
    y0s = [min(y0, VX - YP) for y0 in y0s]
    assert XP * YP * 2 <= 32767, (XP, YP)

    nc = _get_program(XP, YP)

    vol_flat = vol.reshape(-1)
    in_maps = [_prepare_core_inputs(c, depth, vol_flat, Kinv, M, t, org, res,
                                    inv_res, eye_v, XP, YP, x0s, y0s)
               for c in range(8)]

    from concourse.bass_utils import run_bass_kernel_spmd
    res8 = run_bass_kernel_spmd(nc, in_maps, list(range(8)))

    n = H * W
    ext = np.empty((B, n, NPTS), np.float32)
    rp = np.empty((B, n, NPTS, 3), np.float32)
    dirw = np.empty((B, n, 3), np.float32)
    dep = np.empty((B, n), np.float32)
    idxw = np.empty((B, n, NPTS, 3), np.int32)
    crd = np.empty((B, n, 3), np.float32)
    for core in range(8):
        b = core // 4
        r0 = CORE_ROWS * (core % 4)
        sl = slice(r0 * W, (r0 + CORE_ROWS) * W)
        r = res8.results[core]
        ep = r["ext"].reshape(8, 16, NCHUNK, NGROUPS, CALLS_PER_GROUP, COLS8)
        ep = ep.transpose(3, 1, 2, 4, 5, 0).reshape(PROWS, NCHUNK * SAMP_PER_ROW)
        ext[b, sl] = ep[:CORE_ROWS].reshape(CORE_ROWS * W, NPTS)
        rp[b, sl] = r["rp"][:CORE_ROWS].reshape(CORE_ROWS * W, NPTS, 3)
        dirw[b, sl] = r["dirw"][:CORE_ROWS].reshape(CORE_ROWS * W, 3)
        dep[b, sl] = r["dep"][:CORE_ROWS].reshape(CORE_ROWS * W)
        idxw[b, sl] = r["idxw"][:CORE_ROWS].reshape(CORE_ROWS * W, NPTS, 3)
        crd[b, sl] = r["crd"][:CORE_ROWS].reshape(CORE_ROWS * W, 3)
    return ext, rp, dirw, dep, idxw, crd
